# revision 41
# baseline (speedup 1.0000x reference)
"""MDLSTM cell (2-direction) Bass/Tile kernel for Trainium2, 8-core SPMD.

Math (per direction d, with shared input projections):
    i = sigmoid(w_ii @ x + w_hi @ h_d + b_i)
    f = sigmoid(w_if @ x + w_hf @ h_d + b_f)
    g = tanh   (w_ig @ x + w_hg @ h_d + b_g)
    o = sigmoid(w_io @ x + w_ho @ h_d + b_o)
    c_d = f * c_prev_d + i * g
    h_d = o * tanh(c_d)
ct = ws0 * c_0 + ws1 * c_1 ;  ht = ws0 * h_0 + ws1 * h_1

Sharding: all activations/states split along N (=8192) across 8 cores;
weights replicated. No cross-core communication.

Per-core kernel, one n-chunk (512 cols) at a time: per gate the shared
x-projection accumulates in a PSUM bank (start=True group); a DVE
PSUM->PSUM copy clones it into a second bank as direction 1's base, then
direction 0's hidden projection accumulates IN PLACE on the px bank
(start=False matmuls - no inject needed) and direction 1's accumulates on
the copy. For the very first block, direction 1's base is RECOMPUTED
(start=True) instead of copied: only TensorE writes set PSUM has_written
bits, and a start=False accumulate onto a DVE-written virgin bank would
overwrite instead of accumulate - the recompute also fills the DMA-bound
startup with useful work. ScalarE applies sigmoid/tanh + per-partition
bias straight out of PSUM; the elementwise cell update runs on DVE
(GpSimd measured 1.4us/op, and an all-GpSimd variant coincided with a
chip-wide ~1.2x downclock), split per direction and software-pipelined
one block behind the matmuls so the px copies stay near the DVE queue
head (the in-place direction-0 matmuls carry a WAR dependency on them).

Precision: matmul operands are fp16 (same PE rate as bf16, 3 extra
mantissa bits of error headroom) except fp8e4m3 DoubleRow k-tiles (2
k-tiles per instruction, batched into contiguous runs - the first DR
after a non-DR matmul pays ~190ns of exposed LDWEIGHTS, back-to-back DRs
hide it and issue at the same 216ns cadence as one bf16 k-tile).
Per-(direction,gate) fp8 k-tile counts are hardcoded from an offline
ml_dtypes error model (bit-faithful: matched HW rel_fro to 4 digits at
three calibration points); the low-|ws| direction runs fully fp8 - its
quantization error is attenuated by the weighted sum - and the tanh (g)
gate is the most error-sensitive, so the high-|ws| direction keeps it in
fp16. Measured rel_fro 1.874e-2 vs the 2e-2 gate. c_prev loads and ct/ht
stores are fp16 (upcast on host). The direction weighted sum is folded
in algebraically: c_prev is pre-scaled by ws_d on the host, (i*ws)*g
runs as one scalar_tensor_tensor, tanh(c_d) is recovered from the
weighted cw via activation input scale 1/ws_d, and the final combine is
two adds.

Startup is DMA-ramp-bound (~8.5us before the first payload packet):
weights are laid out with all 4 gates contiguous per partition row and
x k-tile 0 rides at the head of the Activation engine's HW-DGE queue
(it ramps ~2us before Sync's); the first block's px runs k-tile-major so
the first 8 matmuls need only half of x. Later weight prefetches go via
Sync; c_prev loads and output stores trigger from the scalar queue -
a DMA trigger blocks its engine queue until the previous transfer on the
same hw queue completes, so small transfers must not share a queue with
the multi-MB weight prefetches.
"""

import numpy as np

import concourse.bass as bass  # noqa: F401  (bass types via bacc/tile)
import concourse.mybir as mybir
import concourse.tile as tile
from concourse import bacc
from concourse.bass_utils import run_bass_kernel_spmd

N_CORES = 8
IN_C = 512
OUT_C = 1024
N = 8192
NS = N // N_CORES  # columns per core
NCH = 512  # psum free-dim chunk (one bank)
N_CHUNKS = NS // NCH
KX = IN_C // 128  # k-tiles of the input projection
KH = OUT_C // 128  # k-tiles of the hidden projection
M_TILES = OUT_C // 128

F32 = mybir.dt.float32
MM_MODE = "fp16"  # one of: "fp32r", "bf16", "fp16"
import ml_dtypes as _mld
MM_DT = {"fp32r": mybir.dt.float32r, "bf16": mybir.dt.bfloat16,
         "fp16": mybir.dt.float16}[MM_MODE]
MM_NP = {"fp32r": np.float32, "bf16": _mld.bfloat16,
         "fp16": np.float16}[MM_MODE]

SIG = mybir.ActivationFunctionType.Sigmoid
TANH = mybir.ActivationFunctionType.Tanh
MULT = mybir.AluOpType.mult
ADD = mybir.AluOpType.add
COPY = mybir.ActivationFunctionType.Copy

# fp8 k-tile counts (must be even - DoubleRow packs 2 k-tiles/instruction).
# _LO applies to the direction with the smaller |ws| (its error is
# attenuated by the weighted sum), _HI to the other. Per-gate order
# [i, f, g, o]. KX8 is the shared x projection (error hits both dirs).
# Chosen by the offline ml_dtypes error model (err_model.py greedy, fp16
# high path): predicted rel_fro 1.875e-2 vs the 2e-2 gate (model matched
# HW to 4 digits at two calibration points; inputs are deterministic).
KF8_LO_G = [8, 8, 8, 8]
KF8_HI_G = [8, 8, 0, 4]
KX8_G = [0, 0, 0, 0]
KF8MAX = max(max(KF8_LO_G), max(KF8_HI_G))
KH_MIN8 = min(min(KF8_LO_G), min(KF8_HI_G))  # bf16 wh skips these k-tiles
KX8MAX = max(KX8_G)
F8 = mybir.dt.float8e4
F8_NP = _mld.float8_e4m3fn
DR = mybir.MatmulPerfMode.DoubleRow


def _build(ws0: float, ws1: float):
    nc = bacc.Bacc(
        "TRN2", target_bir_lowering=False, debug=False, num_devices=N_CORES
    )

    lo_first = abs(ws0) <= abs(ws1)
    kf8_by_dir = [KF8_LO_G, KF8_HI_G] if lo_first else [KF8_HI_G, KF8_LO_G]
    kf8max_d = [max(kf8_by_dir[d]) for d in (0, 1)]
    kf8min_d = [min(kf8_by_dir[d]) for d in (0, 1)]

    xd = nc.dram_tensor("x", [128, KX, NS], MM_DT, kind="ExternalInput")
    x8d = (
        nc.dram_tensor("x8", [128, KX8MAX, NS], F8, kind="ExternalInput")
        if KX8MAX > 0 else None
    )
    # bf16 h only carries the k-tiles the bf16 path actually reads
    # (kf8min_d..KH-1); the first k-tiles live only in the fp8 copies.
    hd_ = [
        nc.dram_tensor(
            f"h{d}", [128, KH - kf8min_d[d], NS], MM_DT, kind="ExternalInput",
        ) if kf8min_d[d] < KH else None
        for d in (0, 1)
    ]
    cd_ = [
        nc.dram_tensor(f"c{d}", [OUT_C, NS], MM_DT, kind="ExternalInput")
        for d in (0, 1)
    ]
    # weights: [m_tile, partition(k%128), gate, k_tile, m_in_tile] - all 4
    # gates contiguous per partition row so one DMA moves 4-8KB runs
    # (startup is DMA packet-rate-bound, not byte-bound). bf16 wh skips
    # k-tiles 0..KH_MIN8-1 (fp8 in every direction/gate).
    wxd = nc.dram_tensor("wx", [M_TILES, 128, 4, KX, 128], MM_DT, kind="ExternalInput")
    wx8d = (
        nc.dram_tensor("wx8", [M_TILES, 128, 4, KX8MAX, 128], F8, kind="ExternalInput")
        if KX8MAX > 0 else None
    )
    whd = nc.dram_tensor(
        "wh", [M_TILES, 128, 4, KH - KH_MIN8, 128], MM_DT, kind="ExternalInput"
    )
    wh8d = (
        nc.dram_tensor(
            "wh8", [M_TILES, 128, 4, KF8MAX, 128], F8, kind="ExternalInput"
        ) if KF8MAX > 0 else None
    )
    h8d_ = [
        nc.dram_tensor(
            f"h8{d}", [128, kf8max_d[d], NS], F8, kind="ExternalInput"
        ) if kf8max_d[d] > 0 else None
        for d in (0, 1)
    ]
    biasd = nc.dram_tensor("bias", [128, 4 * M_TILES], F32, kind="ExternalInput")
    # outputs stored bf16 (upcast on host): halves the store traffic that
    # gates the kernel-end barrier; adds only ~0.2% rms to the outputs
    ctd = nc.dram_tensor("ct", [OUT_C, NS], MM_DT, kind="ExternalOutput")
    htd = nc.dram_tensor("ht", [OUT_C, NS], MM_DT, kind="ExternalOutput")

    # reciprocal scales for the tanh(c) recovery; clamp so ws==0 stays finite
    # (then cw==0 and tanh(0)==0 gives the right answer anyway)
    inv_ws = [1.0 / max(ws0, 1e-20), 1.0 / max(ws1, 1e-20)]
    wss = [ws0, ws1]

    with tile.TileContext(nc) as tc:
        with (
            tc.tile_pool(name="resident", bufs=1) as res_pool,
            tc.tile_pool(name="wx", bufs=3) as wx_pool,
            tc.tile_pool(name="wh", bufs=3) as wh_pool,
            tc.tile_pool(name="wh8", bufs=3) as wh8_pool,
            tc.tile_pool(name="psum", bufs=8, space="PSUM") as ps_pool,
            tc.tile_pool(name="gates", bufs=16) as g_pool,
            tc.tile_pool(name="cprev", bufs=4) as cp_pool,
            tc.tile_pool(name="tmp", bufs=8) as t_pool,
            tc.tile_pool(name="dirres", bufs=6) as dr_pool,
            tc.tile_pool(name="out", bufs=4) as o_pool,
        ):
            wx_tiles: dict = {}
            wx8_tiles: dict = {}
            wh_tiles: dict = {}
            wh8_tiles: dict = {}

            def alloc_w(mt):
                wx_tiles[mt] = wx_pool.tile(
                    [128, 4, KX, 128], MM_DT, tag="wx", name=f"wx_{mt}"
                )
                wh_tiles[mt] = wh_pool.tile(
                    [128, 4, KH - KH_MIN8, 128], MM_DT, tag="wh", name=f"wh_{mt}"
                )
                if wh8d is not None:
                    wh8_tiles[mt] = wh8_pool.tile(
                        [128, 4, KF8MAX, 128], F8, tag="wh8", name=f"wh8_{mt}"
                    )
                if wx8d is not None:
                    wx8_tiles[mt] = wx8_pool.tile(
                        [128, 4, KX8MAX, 128], F8, tag="wx8", name=f"wx8_{mt}"
                    )

            def load_w(mt, eng=None):
                # startup loads ride the Activation engine's HW-DGE queue so
                # the trigger path ramps in parallel with Sync's x/h loads;
                # mid-kernel prefetches go via Sync - a DMA trigger can block
                # its queue, and the scalar queue's ACTs free PSUM banks.
                eng = eng or nc.sync
                alloc_w(mt)
                eng.dma_start(wx_tiles[mt][:], wxd[mt])
                eng.dma_start(wh_tiles[mt][:], whd[mt])
                if wh8d is not None:
                    eng.dma_start(wh8_tiles[mt][:], wh8d[mt])
                if wx8d is not None:
                    eng.dma_start(wx8_tiles[mt][:], wx8d[mt])

            x_sb = res_pool.tile([128, KX, NS], MM_DT, tag="x")
            x8_sb = (
                res_pool.tile([128, KX8MAX, NS], F8, tag="x8")
                if x8d is not None else None
            )
            h_sb = [
                res_pool.tile(
                    [128, KH - kf8min_d[d], NS], MM_DT,
                    tag=f"h{d}", name=f"h_sb{d}",
                ) if hd_[d] is not None else None
                for d in (0, 1)
            ]
            bias_sb = res_pool.tile([128, 4 * M_TILES], F32, tag="bias")

            # Startup is DMA packet-rate-bound: full-tensor loads give
            # 8-16KB contiguous runs (vs 1KB chunked), cutting packet count
            # ~8x. Order by first use; wx0 split per gate-pair so px g0/g1
            # can start before the full gate set lands.
            alloc_w(0)
            # The two tensors MM #1 needs are split ACROSS the two HW-DGE
            # queues so they transfer in parallel (the 16 DMA engines
            # round-robin between queues): x k-tile 0 heads the scalar
            # queue (ramps ~2us before Sync's), wx0 gates 0-1 head Sync's.
            # Measured: both on one queue serialized and gated MM #1 at
            # ~14.9us.
            nc.scalar.dma_start(x_sb[:, 0:1], xd[:, 0:1])
            nc.scalar.dma_start(wx_tiles[0][:, 2:4], wxd[0][:, 2:4])
            if wx8d is not None:
                nc.scalar.dma_start(wx8_tiles[0][:], wx8d[0])
            # wh8 before wh: direction 0 (all-fp8) runs right after px
            if wh8d is not None:
                nc.scalar.dma_start(wh8_tiles[0][:], wh8d[0])
            nc.scalar.dma_start(wh_tiles[0][:], whd[0])
            nc.sync.dma_start(wx_tiles[0][:, 0:2], wxd[0][:, 0:2])
            h8_sb = [
                res_pool.tile(
                    [128, kf8max_d[d], NS], F8,
                    tag=f"h8{d}", name=f"h8_sb{d}",
                ) if h8d_[d] is not None else None
                for d in (0, 1)
            ]
            # x in two k-tile halves (same layout, 4KB runs): the first px
            # matmuls only need k-tiles 0-1, shaving the critical startup
            # bytes before the first MM
            nc.sync.dma_start(x_sb[:, 1:2], xd[:, 1:2])
            nc.sync.dma_start(x_sb[:, 2:3], xd[:, 2:3])
            nc.sync.dma_start(x_sb[:, 3:4], xd[:, 3:4])
            if x8_sb is not None:
                nc.sync.dma_start(x8_sb[:], x8d[:])
            # activation loads ordered by first use: fp8 h first (the all-
            # fp8 direction's DR run follows px immediately), bias before
            # the first gate ACT, the big fp16 h in halves so the first
            # bf16 hidden matmuls need not wait for the full tensor
            if h8_sb[0] is not None:
                nc.sync.dma_start(h8_sb[0][:], h8d_[0][:])
            nc.sync.dma_start(bias_sb[:], biasd[:])
            for d in (0, 1):
                if h_sb[d] is not None:
                    kt_n = KH - kf8min_d[d]
                    if kt_n >= 2:
                        nc.sync.dma_start(h_sb[d][:, : kt_n // 2], hd_[d][:, : kt_n // 2])
                        nc.sync.dma_start(h_sb[d][:, kt_n // 2 :], hd_[d][:, kt_n // 2 :])
                    else:
                        nc.sync.dma_start(h_sb[d][:], hd_[d][:])
                if d == 1 and h8_sb[d] is not None:
                    nc.sync.dma_start(h8_sb[d][:], h8d_[d][:])
            # NOT on the scalar queue: a DMA trigger blocks its queue until
            # the previous transfer on the same hw queue completes, and the
            # scalar queue must stay free for the ACTs the phases wait on.
            load_w(1)

            def px_phase(mt, n, wxm, wx8m, recompute=False):
                """Per gate: x-projection into a PSUM bank (start=True) and
                a DVE clone of it into a second bank (direction 1's base).
                fp8 k-tiles of all gates run as one contiguous DoubleRow run
                (a DR instruction after a non-DR one pays ~183ns of exposed
                LDWEIGHTS; back-to-back DR ones hide it).
                Returns [(px_bank, dir1_bank), ...] per gate."""
                nsl = slice(n * NCH, (n + 1) * NCH)
                banks = []
                for g in range(4):
                    px = ps_pool.tile(
                        [128, NCH], F32, tag="ps", name=f"px_{mt}_{n}_{g}"
                    )
                    banks.append([px, None])
                # First block: k-tile-major, so the first 8 matmuls need
                # only the first half of x (lands first). Elsewhere:
                # gate-major, so gate g's bank completes early and its DVE
                # clone is ready before direction 0 reaches that bank.
                order = (
                    [(kt, g) for kt in range(KX) for g in range(4)]
                    if recompute else
                    [(kt, g) for g in range(4) for kt in range(KX)]
                )
                for kt, g in order:
                    kx8 = KX8_G[g]
                    if kt < kx8:
                        continue
                    nc.tensor.matmul(
                        banks[g][0][:],
                        wxm[:, g, kt, :],
                        x_sb[:, kt, nsl],
                        start=(kt == kx8),
                        stop=(kt == KX - 1 and kx8 == 0),
                    )
                for g in range(4):  # px DoubleRow run
                    kx8 = KX8_G[g]
                    for p in range(kx8 // 2):
                        nc.tensor.matmul(
                            banks[g][0][:],
                            wx8m[:, g, 2 * p : 2 * p + 2, :],
                            x8_sb[:, 2 * p : 2 * p + 2, nsl],
                            start=(kx8 == KX and p == 0),
                            stop=(p == kx8 // 2 - 1),
                            skip_group_check=True,
                            perf_mode=DR,
                        )
                for g in range(4):
                    b1 = ps_pool.tile(
                        [128, NCH], F32, tag="ps", name=f"b1_{mt}_{n}_{g}"
                    )
                    if recompute:
                        # First block only: recompute the x-projection into
                        # direction 1's banks instead of cloning. This puts a
                        # start=True group on every PSUM bank before any
                        # DVE-write + start=False accumulate happens (only
                        # TensorE writes set the has_written bits; without
                        # them those matmuls would overwrite, not accumulate)
                        # and fills the DMA-bound startup with useful work.
                        for kt in range(KX):
                            nc.tensor.matmul(
                                b1[:],
                                wxm[:, g, kt, :],
                                x_sb[:, kt, nsl],
                                start=(kt == 0),
                                stop=(kt == KX - 1),
                            )
                    else:
                        # high_priority: the Tile scheduler orders engine
                        # queues by its own heap; the copies must land ahead
                        # of the previous block's elementwise on DVE or the
                        # in-place direction-0 matmuls (WAR on the copy
                        # read) stall ~0.6us per block
                        with tc.high_priority():
                            nc.vector.tensor_copy(b1[:], banks[g][0][:])
                    banks[g][1] = b1
                return banks

            def dir_mms(mt, n, d, banks, whm, wh8m):
                """Hidden-projection accumulation for one direction onto the
                already-seeded PSUM banks (bank index: 0 = in place on px,
                1 = the DVE clone), then the gate activations. All gates'
                bf16 k-tiles first, then every fp8 pair in one contiguous
                DoubleRow run; each gate's ACT follows its last matmul."""
                nsl = slice(n * NCH, (n + 1) * NCH)
                gt = [None] * 4

                def act(g):
                    ps = banks[g][0 if d == 0 else 1]
                    gact = g_pool.tile(
                        [128, NCH], MM_DT, tag="gate", name=f"gate_{mt}_{n}_{d}_{g}"
                    )
                    nc.scalar.activation(
                        gact[:],
                        ps[:],
                        TANH if g == 2 else SIG,
                        bias=bias_sb[:, g * M_TILES + mt : g * M_TILES + mt + 1],
                    )
                    gt[g] = gact

                for g in range(4):
                    ps = banks[g][0 if d == 0 else 1]
                    kf8 = kf8_by_dir[d][g]
                    for kh in range(kf8, KH):
                        nc.tensor.matmul(
                            ps[:],
                            whm[:, g, kh - KH_MIN8, :],
                            h_sb[d][:, kh - kf8min_d[d], nsl],
                            start=False,
                            stop=(kf8 == 0 and kh == KH - 1),
                            skip_group_check=True,
                        )
                    if kf8 == 0:
                        act(g)
                for g in range(4):  # hidden DoubleRow run
                    kf8 = kf8_by_dir[d][g]
                    if kf8 == 0:
                        continue
                    for p in range(kf8 // 2):
                        nc.tensor.matmul(
                            ps := banks[g][0 if d == 0 else 1],
                            wh8m[:, g, 2 * p : 2 * p + 2, :],
                            h8_sb[d][:, 2 * p : 2 * p + 2, nsl],
                            start=False,
                            stop=(p == kf8 // 2 - 1),
                            skip_group_check=True,
                            perf_mode=DR,
                        )
                    act(g)
                return gt

            def tail_dir(mt, n, d, gt, cpd):
                """Cell-update front half for one direction: ig/fc/cw on
                DVE, tanh on ScalarE. Direction 0's runs mid-block (its
                gates finish after dir0's matmuls); direction 1's runs at
                the START of the next block - only 3 DVE ops then sit ahead
                of the px copies, which stay inside the px window. (GpSimd
                stays idle: 1.4us/op, and an early all-GpSimd variant
                coincided with a chip-wide ~1.2x downclock.)"""
                ws, iws = wss[d], inv_ws[d]
                ig = t_pool.tile([128, NCH], MM_DT, tag="ig", name=f"ig{d}")
                nc.vector.scalar_tensor_tensor(ig[:], gt[0][:], ws, gt[2][:], MULT, MULT)
                fc = t_pool.tile([128, NCH], MM_DT, tag="fc", name=f"fc{d}")
                nc.vector.tensor_mul(fc[:], gt[1][:], cpd[:])
                cwd = dr_pool.tile([128, NCH], MM_DT, tag="cw", name=f"cw{d}")
                nc.vector.tensor_add(cwd[:], ig[:], fc[:])
                tchd = t_pool.tile([128, NCH], MM_DT, tag="tch", name=f"tch{d}")
                nc.scalar.activation(tchd[:], cwd[:], TANH, scale=iws)
                return cwd, tchd

            def tail_fin(mt, n, gt_d, cp, msl, cw, tch):
                """Cell-update back half: hw, direction combine, stores."""
                nsl = slice(n * NCH, (n + 1) * NCH)
                hw = []
                for d in (0, 1):
                    hwd = dr_pool.tile([128, NCH], MM_DT, tag="hw", name=f"hw{d}")
                    nc.vector.scalar_tensor_tensor(
                        hwd[:], gt_d[d][3][:], wss[d], tch[d][:], MULT, MULT
                    )
                    hw.append(hwd)
                # combine + store; output triggers ride the scalar queue
                ctt = o_pool.tile([128, NCH], MM_DT, tag="ctt")
                nc.vector.tensor_add(ctt[:], cw[0][:], cw[1][:])
                nc.scalar.dma_start(ctd[msl, nsl], ctt[:])
                htt = o_pool.tile([128, NCH], MM_DT, tag="htt")
                nc.vector.tensor_add(htt[:], hw[0][:], hw[1][:])
                nc.scalar.dma_start(htd[msl, nsl], htt[:])

            pending = None  # (mt, n, gt_d, cp, msl, cw0, tch0)
            for mt in range(M_TILES):
                msl = slice(mt * 128, (mt + 1) * 128)
                if mt + 2 < M_TILES:
                    load_w(mt + 2)
                wxm = wx_tiles.pop(mt)
                whm = wh_tiles.pop(mt)
                wh8m = wh8_tiles.pop(mt) if wh8d is not None else None
                wx8m = wx8_tiles.pop(mt) if wx8d is not None else None

                for n in range(N_CHUNKS):
                    # c_prev loads (host pre-scaled by ws_d) ride the SCALAR
                    # hw queue (small store transfers only): on Sync they
                    # trigger behind the multi-MB weight prefetches, land
                    # ~10us late, and their wait then head-of-line-blocks
                    # the DVE FIFO ahead of the px copies
                    nsl = slice(n * NCH, (n + 1) * NCH)
                    cp = [
                        cp_pool.tile([128, NCH], MM_DT, tag="cp", name=f"cp_{mt}_{n}_{d}")
                        for d in (0, 1)
                    ]
                    nc.scalar.dma_start(cp[0][:], cd_[0][msl, nsl])
                    nc.scalar.dma_start(cp[1][:], cd_[1][msl, nsl])

                    banks = px_phase(mt, n, wxm, wx8m, recompute=(mt == 0 and n == 0))
                    if pending is not None:
                        p_mt, p_n, p_gt, p_cp, p_msl, p_cw0, p_tch0 = pending
                        # NOTE: the ~0.6us/block stall of the first in-place
                        # direction-0 matmul (its copy WAR wait is hoisted
                        # onto prefetched LDWEIGHTS, so it effectively waits
                        # for the g1 copy too) resists reordering: the
                        # scheduler is work-conserving and dispatches this
                        # elementwise (ready at block start) ahead of the
                        # copies; tc.high_priority() doesn't override
                        # readiness order and tc.tile_wait_until() emits
                        # real runtime waits (measured +7us). Accepted.
                        cw1, tch1 = tail_dir(p_mt, p_n, 1, p_gt[1], p_cp[1])
                    # direction 0 in place on the px banks, 1 on the clones
                    gt0 = dir_mms(mt, n, 0, banks, whm, wh8m)
                    cw0, tch0 = tail_dir(mt, n, 0, gt0, cp[0])
                    gt1 = dir_mms(mt, n, 1, banks, whm, wh8m)
                    if pending is not None:
                        tail_fin(p_mt, p_n, p_gt, p_cp, p_msl,
                                 [p_cw0, cw1], [p_tch0, tch1])
                    pending = (mt, n, [gt0, gt1], cp, msl, cw0, tch0)
            # Final block's tail at HALF width, two pipelined halves: this
            # chain runs after the last matmul and is latency-bound across
            # alternating DVE/ScalarE hops, so halving the op width shortens
            # the critical path (half 1's vector ops overlap half 0's tanh).
            p_mt, p_n, p_gt, p_cp, p_msl, p_cw0, p_tch0 = pending
            ws1_, iws1 = wss[1], inv_ws[1]
            NH = NCH // 2
            for hi in range(2):
                h = slice(hi * NH, (hi + 1) * NH)
                nslh = slice(p_n * NCH + hi * NH, p_n * NCH + (hi + 1) * NH)
                gt1 = p_gt[1]
                ig = t_pool.tile([128, NH], MM_DT, tag="ig", name=f"figh{hi}")
                nc.vector.scalar_tensor_tensor(
                    ig[:], gt1[0][:, h], ws1_, gt1[2][:, h], MULT, MULT
                )
                fc = t_pool.tile([128, NH], MM_DT, tag="fc", name=f"ffch{hi}")
                nc.vector.tensor_mul(fc[:], gt1[1][:, h], p_cp[1][:, h])
                cw1 = dr_pool.tile([128, NH], MM_DT, tag="cw", name=f"fcwh{hi}")
                nc.vector.tensor_add(cw1[:], ig[:], fc[:])
                tch1 = t_pool.tile([128, NH], MM_DT, tag="tch", name=f"ftchh{hi}")
                nc.scalar.activation(tch1[:], cw1[:], TANH, scale=iws1)
                hw0 = dr_pool.tile([128, NH], MM_DT, tag="hw", name=f"fhw0h{hi}")
                nc.vector.scalar_tensor_tensor(
                    hw0[:], p_gt[0][3][:, h], wss[0], p_tch0[:, h], MULT, MULT
                )
                hw1 = dr_pool.tile([128, NH], MM_DT, tag="hw", name=f"fhw1h{hi}")
                nc.vector.scalar_tensor_tensor(
                    hw1[:], gt1[3][:, h], ws1_, tch1[:], MULT, MULT
                )
                ctt = o_pool.tile([128, NH], MM_DT, tag="ctt", name=f"fctth{hi}")
                nc.vector.tensor_add(ctt[:], p_cw0[:, h], cw1[:])
                nc.scalar.dma_start(ctd[p_msl, nslh], ctt[:])
                htt = o_pool.tile([128, NH], MM_DT, tag="htt", name=f"fhtth{hi}")
                nc.vector.tensor_add(htt[:], hw0[:], hw1[:])
                nc.scalar.dma_start(htd[p_msl, nslh], htt[:])

    nc.finalize()
    n_mm = sum(
        1 for i in nc.inst_map.values() if type(i).__name__ == "InstMatmult"
    )
    expected_mm = 4 * KX + M_TILES * N_CHUNKS * sum(
        (KX - KX8_G[g]) + KX8_G[g] // 2
        + sum((KH - kf8_by_dir[d][g]) + kf8_by_dir[d][g] // 2 for d in (0, 1))
        for g in range(4)
    )
    assert n_mm == expected_mm, f"matmul count {n_mm} != {expected_mm}"
    return nc


_CACHE: dict = {}


def _get_nc(ws0: float, ws1: float):
    key = (ws0, ws1)
    if key not in _CACHE:
        _CACHE.clear()
        _CACHE[key] = _build(ws0, ws1)
    return _CACHE[key]


def _prep_w(w: np.ndarray, ktiles, np_dt) -> np.ndarray:
    """(OUT_C, K) weight -> [m_tile, partition, k_tile, m_in_tile] lhsT tiles
    for the given k-tile indices."""
    wT = np.ascontiguousarray(w.T)  # (K, OUT_C)
    r = wT.reshape(-1, 128, M_TILES, 128)  # [ktile, p, mtile, mi]
    r = r[list(ktiles)]
    return np.ascontiguousarray(r.transpose(2, 1, 0, 3).astype(np_dt))


def _prep_wstack(ws: list[np.ndarray], ktiles, np_dt) -> np.ndarray:
    """4 gate weights -> [m_tile, partition, gate, k_tile, m_in_tile]."""
    s = np.stack([_prep_w(w, ktiles, np_dt) for w in ws])  # [g, mt, p, kt, mi]
    return np.ascontiguousarray(s.transpose(1, 2, 0, 3, 4))


def _prep_rhs(a: np.ndarray, k0: int, k1: int, np_dt) -> np.ndarray:
    """(K, n) activation k-tiles [k0,k1) -> [partition, k_tile, n]."""
    r = a[k0 * 128 : k1 * 128].reshape(k1 - k0, 128, -1).transpose(1, 0, 2)
    return np.ascontiguousarray(r.astype(np_dt))


def run(inputs: dict, trace: bool = False, trace_kwargs: dict | None = None):
    x = np.asarray(inputs["x"], dtype=np.float32)
    ws = np.asarray(inputs["weighted_sum"], dtype=np.float32)
    ws0, ws1 = float(ws[0]), float(ws[1])
    nc = _get_nc(ws0, ws1)

    lo_first = abs(ws0) <= abs(ws1)
    kf8_by_dir = [KF8_LO_G, KF8_HI_G] if lo_first else [KF8_HI_G, KF8_LO_G]
    kf8max_d = [max(kf8_by_dir[d]) for d in (0, 1)]
    kf8min_d = [min(kf8_by_dir[d]) for d in (0, 1)]

    wx_list = [np.asarray(inputs[k], dtype=np.float32)
               for k in ("w_ii", "w_if", "w_ig", "w_io")]
    wh_list = [np.asarray(inputs[k], dtype=np.float32)
               for k in ("w_hi", "w_hf", "w_hg", "w_ho")]
    wx_host = _prep_wstack(wx_list, range(KX), MM_NP)
    wh_host = _prep_wstack(wh_list, range(KH_MIN8, KH), MM_NP)
    wh8_host = (
        _prep_wstack(wh_list, range(KF8MAX), F8_NP) if KF8MAX > 0 else None
    )
    wx8_host = (
        _prep_wstack(wx_list, range(KX8MAX), F8_NP) if KX8MAX > 0 else None
    )
    bias_host = np.concatenate(
        [np.asarray(inputs[k], dtype=np.float32).reshape(M_TILES, 128).T
         for k in ("b_i", "b_f", "b_g", "b_o")],
        axis=1,
    )
    bias_host = np.ascontiguousarray(bias_host)

    h = [np.asarray(inputs["h_prev_dim0"], dtype=np.float32),
         np.asarray(inputs["h_prev_dim1"], dtype=np.float32)]
    # c_prev is pre-scaled by the direction weight on the host; the kernel
    # computes cw_d = ws_d*c_d directly and ct = cw_0 + cw_1.
    c = [(np.asarray(inputs["c_prev_dim0"], dtype=np.float32) * ws0).astype(MM_NP),
         (np.asarray(inputs["c_prev_dim1"], dtype=np.float32) * ws1).astype(MM_NP)]

    in_maps = []
    for core in range(N_CORES):
        csl = slice(core * NS, (core + 1) * NS)
        m = {
            "x": _prep_rhs(x[:, csl], 0, KX, MM_NP),
            "c0": np.ascontiguousarray(c[0][:, csl]),
            "c1": np.ascontiguousarray(c[1][:, csl]),
            "wx": wx_host,
            "wh": wh_host,
            "bias": bias_host,
        }
        if wh8_host is not None:
            m["wh8"] = wh8_host
        if wx8_host is not None:
            m["wx8"] = wx8_host
            m["x8"] = _prep_rhs(x[:, csl], 0, KX8MAX, F8_NP)
        for d in (0, 1):
            if kf8min_d[d] < KH:
                m[f"h{d}"] = _prep_rhs(h[d][:, csl], kf8min_d[d], KH, MM_NP)
            if kf8max_d[d] > 0:
                m[f"h8{d}"] = _prep_rhs(h[d][:, csl], 0, kf8max_d[d], F8_NP)
        in_maps.append(m)

    res = run_bass_kernel_spmd(
        nc,
        in_maps,
        list(range(N_CORES)),
        trace=trace,
        **(trace_kwargs or {}),
    )
    ct = np.concatenate(
        [res.results[c]["ct"].astype(np.float32) for c in range(N_CORES)], axis=1
    )
    ht = np.concatenate(
        [res.results[c]["ht"].astype(np.float32) for c in range(N_CORES)], axis=1
    )
    return (ct, ht), res


def kernel(**inputs) -> tuple:
    (ct, ht), _ = run(inputs)
    return ct, ht


# revision 42
# speedup vs baseline: 1.0007x; 1.0007x over previous
"""MDLSTM cell (2-direction) Bass/Tile kernel for Trainium2, 8-core SPMD.

Math (per direction d, with shared input projections):
    i = sigmoid(w_ii @ x + w_hi @ h_d + b_i)
    f = sigmoid(w_if @ x + w_hf @ h_d + b_f)
    g = tanh   (w_ig @ x + w_hg @ h_d + b_g)
    o = sigmoid(w_io @ x + w_ho @ h_d + b_o)
    c_d = f * c_prev_d + i * g
    h_d = o * tanh(c_d)
ct = ws0 * c_0 + ws1 * c_1 ;  ht = ws0 * h_0 + ws1 * h_1

Sharding: all activations/states split along N (=8192) across 8 cores;
weights replicated. No cross-core communication.

Per-core kernel, one n-chunk (512 cols) at a time: per gate the shared
x-projection accumulates in a PSUM bank (start=True group); a DVE
PSUM->PSUM copy clones it into a second bank as direction 1's base, then
direction 0's hidden projection accumulates IN PLACE on the px bank
(start=False matmuls - no inject needed) and direction 1's accumulates on
the copy. For the very first block, direction 1's base is RECOMPUTED
(start=True) instead of copied: only TensorE writes set PSUM has_written
bits, and a start=False accumulate onto a DVE-written virgin bank would
overwrite instead of accumulate - the recompute also fills the DMA-bound
startup with useful work. ScalarE applies sigmoid/tanh + per-partition
bias straight out of PSUM; the elementwise cell update runs on DVE
(GpSimd measured 1.4us/op, and an all-GpSimd variant coincided with a
chip-wide ~1.2x downclock), split per direction and software-pipelined
one block behind the matmuls so the px copies stay near the DVE queue
head (the in-place direction-0 matmuls carry a WAR dependency on them).

Precision: matmul operands are fp16 (same PE rate as bf16, 3 extra
mantissa bits of error headroom) except fp8e4m3 DoubleRow k-tiles (2
k-tiles per instruction, batched into contiguous runs - the first DR
after a non-DR matmul pays ~190ns of exposed LDWEIGHTS, back-to-back DRs
hide it and issue at the same 216ns cadence as one bf16 k-tile).
Per-(direction,gate) fp8 k-tile counts are hardcoded from an offline
ml_dtypes error model (bit-faithful: matched HW rel_fro to 4 digits at
three calibration points); the low-|ws| direction runs fully fp8 - its
quantization error is attenuated by the weighted sum - and the tanh (g)
gate is the most error-sensitive, so the high-|ws| direction keeps it in
fp16. Measured rel_fro 1.874e-2 vs the 2e-2 gate. c_prev loads and ct/ht
stores are fp16 (upcast on host). The direction weighted sum is folded
in algebraically: c_prev is pre-scaled by ws_d on the host, (i*ws)*g
runs as one scalar_tensor_tensor, tanh(c_d) is recovered from the
weighted cw via activation input scale 1/ws_d, and the final combine is
two adds.

Startup is DMA-ramp-bound (~8.5us before the first payload packet):
weights are laid out with all 4 gates contiguous per partition row and
x k-tile 0 rides at the head of the Activation engine's HW-DGE queue
(it ramps ~2us before Sync's); the first block's px runs k-tile-major so
the first 8 matmuls need only half of x. Later weight prefetches go via
Sync; c_prev loads and output stores trigger from the scalar queue -
a DMA trigger blocks its engine queue until the previous transfer on the
same hw queue completes, so small transfers must not share a queue with
the multi-MB weight prefetches.
"""

import numpy as np

import concourse.bass as bass  # noqa: F401  (bass types via bacc/tile)
import concourse.mybir as mybir
import concourse.tile as tile
from concourse import bacc
from concourse.bass_utils import run_bass_kernel_spmd

N_CORES = 8
IN_C = 512
OUT_C = 1024
N = 8192
NS = N // N_CORES  # columns per core
NCH = 512  # psum free-dim chunk (one bank)
N_CHUNKS = NS // NCH
KX = IN_C // 128  # k-tiles of the input projection
KH = OUT_C // 128  # k-tiles of the hidden projection
M_TILES = OUT_C // 128

F32 = mybir.dt.float32
MM_MODE = "fp16"  # one of: "fp32r", "bf16", "fp16"
import ml_dtypes as _mld
MM_DT = {"fp32r": mybir.dt.float32r, "bf16": mybir.dt.bfloat16,
         "fp16": mybir.dt.float16}[MM_MODE]
MM_NP = {"fp32r": np.float32, "bf16": _mld.bfloat16,
         "fp16": np.float16}[MM_MODE]

SIG = mybir.ActivationFunctionType.Sigmoid
TANH = mybir.ActivationFunctionType.Tanh
MULT = mybir.AluOpType.mult
ADD = mybir.AluOpType.add
COPY = mybir.ActivationFunctionType.Copy

# fp8 k-tile counts (must be even - DoubleRow packs 2 k-tiles/instruction).
# _LO applies to the direction with the smaller |ws| (its error is
# attenuated by the weighted sum), _HI to the other. Per-gate order
# [i, f, g, o]. KX8 is the shared x projection (error hits both dirs).
# Chosen by the offline ml_dtypes error model (err_model.py greedy, fp16
# high path): predicted rel_fro 1.875e-2 vs the 2e-2 gate (model matched
# HW to 4 digits at two calibration points; inputs are deterministic).
KF8_LO_G = [8, 8, 8, 8]
KF8_HI_G = [8, 8, 0, 4]
KX8_G = [0, 0, 0, 0]
KF8MAX = max(max(KF8_LO_G), max(KF8_HI_G))
KH_MIN8 = min(min(KF8_LO_G), min(KF8_HI_G))  # bf16 wh skips these k-tiles
KX8MAX = max(KX8_G)
F8 = mybir.dt.float8e4
F8_NP = _mld.float8_e4m3fn
DR = mybir.MatmulPerfMode.DoubleRow


def _build(ws0: float, ws1: float):
    nc = bacc.Bacc(
        "TRN2", target_bir_lowering=False, debug=False, num_devices=N_CORES
    )

    lo_first = abs(ws0) <= abs(ws1)
    kf8_by_dir = [KF8_LO_G, KF8_HI_G] if lo_first else [KF8_HI_G, KF8_LO_G]
    kf8max_d = [max(kf8_by_dir[d]) for d in (0, 1)]
    kf8min_d = [min(kf8_by_dir[d]) for d in (0, 1)]

    xd = nc.dram_tensor("x", [128, KX, NS], MM_DT, kind="ExternalInput")
    x8d = (
        nc.dram_tensor("x8", [128, KX8MAX, NS], F8, kind="ExternalInput")
        if KX8MAX > 0 else None
    )
    # bf16 h only carries the k-tiles the bf16 path actually reads
    # (kf8min_d..KH-1); the first k-tiles live only in the fp8 copies.
    hd_ = [
        nc.dram_tensor(
            f"h{d}", [128, KH - kf8min_d[d], NS], MM_DT, kind="ExternalInput",
        ) if kf8min_d[d] < KH else None
        for d in (0, 1)
    ]
    cd_ = [
        nc.dram_tensor(f"c{d}", [OUT_C, NS], MM_DT, kind="ExternalInput")
        for d in (0, 1)
    ]
    # weights: [m_tile, partition(k%128), gate, k_tile, m_in_tile] - all 4
    # gates contiguous per partition row so one DMA moves 4-8KB runs
    # (startup is DMA packet-rate-bound, not byte-bound). bf16 wh skips
    # k-tiles 0..KH_MIN8-1 (fp8 in every direction/gate).
    wxd = nc.dram_tensor("wx", [M_TILES, 128, 4, KX, 128], MM_DT, kind="ExternalInput")
    wx8d = (
        nc.dram_tensor("wx8", [M_TILES, 128, 4, KX8MAX, 128], F8, kind="ExternalInput")
        if KX8MAX > 0 else None
    )
    whd = nc.dram_tensor(
        "wh", [M_TILES, 128, 4, KH - KH_MIN8, 128], MM_DT, kind="ExternalInput"
    )
    wh8d = (
        nc.dram_tensor(
            "wh8", [M_TILES, 128, 4, KF8MAX, 128], F8, kind="ExternalInput"
        ) if KF8MAX > 0 else None
    )
    h8d_ = [
        nc.dram_tensor(
            f"h8{d}", [128, kf8max_d[d], NS], F8, kind="ExternalInput"
        ) if kf8max_d[d] > 0 else None
        for d in (0, 1)
    ]
    biasd = nc.dram_tensor("bias", [128, 4 * M_TILES], F32, kind="ExternalInput")
    # outputs stored bf16 (upcast on host): halves the store traffic that
    # gates the kernel-end barrier; adds only ~0.2% rms to the outputs
    ctd = nc.dram_tensor("ct", [OUT_C, NS], MM_DT, kind="ExternalOutput")
    htd = nc.dram_tensor("ht", [OUT_C, NS], MM_DT, kind="ExternalOutput")

    # reciprocal scales for the tanh(c) recovery; clamp so ws==0 stays finite
    # (then cw==0 and tanh(0)==0 gives the right answer anyway)
    inv_ws = [1.0 / max(ws0, 1e-20), 1.0 / max(ws1, 1e-20)]
    wss = [ws0, ws1]

    with tile.TileContext(nc) as tc:
        with (
            tc.tile_pool(name="resident", bufs=1) as res_pool,
            tc.tile_pool(name="wx", bufs=3) as wx_pool,
            tc.tile_pool(name="wh", bufs=3) as wh_pool,
            tc.tile_pool(name="wh8", bufs=3) as wh8_pool,
            tc.tile_pool(name="psum", bufs=8, space="PSUM") as ps_pool,
            tc.tile_pool(name="gates", bufs=16) as g_pool,
            tc.tile_pool(name="cprev", bufs=4) as cp_pool,
            tc.tile_pool(name="tmp", bufs=8) as t_pool,
            tc.tile_pool(name="dirres", bufs=6) as dr_pool,
            tc.tile_pool(name="out", bufs=4) as o_pool,
        ):
            wx_tiles: dict = {}
            wx8_tiles: dict = {}
            wh_tiles: dict = {}
            wh8_tiles: dict = {}

            def alloc_w(mt):
                wx_tiles[mt] = wx_pool.tile(
                    [128, 4, KX, 128], MM_DT, tag="wx", name=f"wx_{mt}"
                )
                wh_tiles[mt] = wh_pool.tile(
                    [128, 4, KH - KH_MIN8, 128], MM_DT, tag="wh", name=f"wh_{mt}"
                )
                if wh8d is not None:
                    wh8_tiles[mt] = wh8_pool.tile(
                        [128, 4, KF8MAX, 128], F8, tag="wh8", name=f"wh8_{mt}"
                    )
                if wx8d is not None:
                    wx8_tiles[mt] = wx8_pool.tile(
                        [128, 4, KX8MAX, 128], F8, tag="wx8", name=f"wx8_{mt}"
                    )

            def load_w(mt, eng=None):
                # startup loads ride the Activation engine's HW-DGE queue so
                # the trigger path ramps in parallel with Sync's x/h loads;
                # mid-kernel prefetches go via Sync - a DMA trigger can block
                # its queue, and the scalar queue's ACTs free PSUM banks.
                eng = eng or nc.sync
                alloc_w(mt)
                eng.dma_start(wx_tiles[mt][:], wxd[mt])
                eng.dma_start(wh_tiles[mt][:], whd[mt])
                if wh8d is not None:
                    eng.dma_start(wh8_tiles[mt][:], wh8d[mt])
                if wx8d is not None:
                    eng.dma_start(wx8_tiles[mt][:], wx8d[mt])

            x_sb = res_pool.tile([128, KX, NS], MM_DT, tag="x")
            x8_sb = (
                res_pool.tile([128, KX8MAX, NS], F8, tag="x8")
                if x8d is not None else None
            )
            h_sb = [
                res_pool.tile(
                    [128, KH - kf8min_d[d], NS], MM_DT,
                    tag=f"h{d}", name=f"h_sb{d}",
                ) if hd_[d] is not None else None
                for d in (0, 1)
            ]
            bias_sb = res_pool.tile([128, 4 * M_TILES], F32, tag="bias")

            # Startup is DMA packet-rate-bound: full-tensor loads give
            # 8-16KB contiguous runs (vs 1KB chunked), cutting packet count
            # ~8x. Order by first use; wx0 split per gate-pair so px g0/g1
            # can start before the full gate set lands.
            alloc_w(0)
            # The two tensors MM #1 needs are split ACROSS the two HW-DGE
            # queues so they transfer in parallel (the 16 DMA engines
            # round-robin between queues): x k-tile 0 heads the scalar
            # queue (ramps ~2us before Sync's), wx0 gates 0-1 head Sync's.
            # Measured: both on one queue serialized and gated MM #1 at
            # ~14.9us.
            nc.scalar.dma_start(x_sb[:, 0:1], xd[:, 0:1])
            nc.scalar.dma_start(wx_tiles[0][:, 2:4], wxd[0][:, 2:4])
            if wx8d is not None:
                nc.scalar.dma_start(wx8_tiles[0][:], wx8d[0])
            # wh8 before wh: direction 0 (all-fp8) runs right after px
            if wh8d is not None:
                nc.scalar.dma_start(wh8_tiles[0][:], wh8d[0])
            nc.scalar.dma_start(wh_tiles[0][:], whd[0])
            nc.sync.dma_start(wx_tiles[0][:, 0:2], wxd[0][:, 0:2])
            h8_sb = [
                res_pool.tile(
                    [128, kf8max_d[d], NS], F8,
                    tag=f"h8{d}", name=f"h8_sb{d}",
                ) if h8d_[d] is not None else None
                for d in (0, 1)
            ]
            # x in two k-tile halves (same layout, 4KB runs): the first px
            # matmuls only need k-tiles 0-1, shaving the critical startup
            # bytes before the first MM
            nc.sync.dma_start(x_sb[:, 1:2], xd[:, 1:2])
            nc.sync.dma_start(x_sb[:, 2:3], xd[:, 2:3])
            nc.sync.dma_start(x_sb[:, 3:4], xd[:, 3:4])
            if x8_sb is not None:
                nc.sync.dma_start(x8_sb[:], x8d[:])
            # activation loads ordered by first use: fp8 h first (the all-
            # fp8 direction's DR run follows px immediately), bias before
            # the first gate ACT, the big fp16 h in halves so the first
            # bf16 hidden matmuls need not wait for the full tensor
            if h8_sb[0] is not None:
                nc.sync.dma_start(h8_sb[0][:], h8d_[0][:])
            nc.sync.dma_start(bias_sb[:], biasd[:])
            for d in (0, 1):
                if h_sb[d] is not None:
                    kt_n = KH - kf8min_d[d]
                    if kt_n >= 2:
                        nc.sync.dma_start(h_sb[d][:, : kt_n // 2], hd_[d][:, : kt_n // 2])
                        nc.sync.dma_start(h_sb[d][:, kt_n // 2 :], hd_[d][:, kt_n // 2 :])
                    else:
                        nc.sync.dma_start(h_sb[d][:], hd_[d][:])
                if d == 1 and h8_sb[d] is not None:
                    nc.sync.dma_start(h8_sb[d][:], h8d_[d][:])
            # NOT on the scalar queue: a DMA trigger blocks its queue until
            # the previous transfer on the same hw queue completes, and the
            # scalar queue must stay free for the ACTs the phases wait on.
            load_w(1)

            def px_phase(mt, n, wxm, wx8m, recompute=False):
                """Per gate: x-projection into a PSUM bank (start=True) and
                a DVE clone of it into a second bank (direction 1's base).
                fp8 k-tiles of all gates run as one contiguous DoubleRow run
                (a DR instruction after a non-DR one pays ~183ns of exposed
                LDWEIGHTS; back-to-back DR ones hide it).
                Returns [(px_bank, dir1_bank), ...] per gate."""
                nsl = slice(n * NCH, (n + 1) * NCH)
                banks = []
                for g in range(4):
                    px = ps_pool.tile(
                        [128, NCH], F32, tag="ps", name=f"px_{mt}_{n}_{g}"
                    )
                    banks.append([px, None])
                # First block: k-tile-major, so the first 8 matmuls need
                # only the first half of x (lands first). Elsewhere:
                # gate-major, so gate g's bank completes early and its DVE
                # clone is ready before direction 0 reaches that bank.
                order = (
                    [(kt, g) for kt in range(KX) for g in range(4)]
                    if recompute else
                    [(kt, g) for g in range(4) for kt in range(KX)]
                )
                for kt, g in order:
                    kx8 = KX8_G[g]
                    if kt < kx8:
                        continue
                    nc.tensor.matmul(
                        banks[g][0][:],
                        wxm[:, g, kt, :],
                        x_sb[:, kt, nsl],
                        start=(kt == kx8),
                        stop=(kt == KX - 1 and kx8 == 0),
                    )
                for g in range(4):  # px DoubleRow run
                    kx8 = KX8_G[g]
                    for p in range(kx8 // 2):
                        nc.tensor.matmul(
                            banks[g][0][:],
                            wx8m[:, g, 2 * p : 2 * p + 2, :],
                            x8_sb[:, 2 * p : 2 * p + 2, nsl],
                            start=(kx8 == KX and p == 0),
                            stop=(p == kx8 // 2 - 1),
                            skip_group_check=True,
                            perf_mode=DR,
                        )
                for g in range(4):
                    b1 = ps_pool.tile(
                        [128, NCH], F32, tag="ps", name=f"b1_{mt}_{n}_{g}"
                    )
                    if recompute:
                        # First block only: recompute the x-projection into
                        # direction 1's banks instead of cloning. This puts a
                        # start=True group on every PSUM bank before any
                        # DVE-write + start=False accumulate happens (only
                        # TensorE writes set the has_written bits; without
                        # them those matmuls would overwrite, not accumulate)
                        # and fills the DMA-bound startup with useful work.
                        for kt in range(KX):
                            nc.tensor.matmul(
                                b1[:],
                                wxm[:, g, kt, :],
                                x_sb[:, kt, nsl],
                                start=(kt == 0),
                                stop=(kt == KX - 1),
                            )
                    else:
                        # high_priority: the Tile scheduler orders engine
                        # queues by its own heap; the copies must land ahead
                        # of the previous block's elementwise on DVE or the
                        # in-place direction-0 matmuls (WAR on the copy
                        # read) stall ~0.6us per block
                        with tc.high_priority():
                            nc.vector.tensor_copy(b1[:], banks[g][0][:])
                    banks[g][1] = b1
                return banks

            def dir_mms(mt, n, d, banks, whm, wh8m):
                """Hidden-projection accumulation for one direction onto the
                already-seeded PSUM banks (bank index: 0 = in place on px,
                1 = the DVE clone), then the gate activations. All gates'
                bf16 k-tiles first, then every fp8 pair in one contiguous
                DoubleRow run; each gate's ACT follows its last matmul."""
                nsl = slice(n * NCH, (n + 1) * NCH)
                gt = [None] * 4

                def act(g):
                    ps = banks[g][0 if d == 0 else 1]
                    gact = g_pool.tile(
                        [128, NCH], MM_DT, tag="gate", name=f"gate_{mt}_{n}_{d}_{g}"
                    )
                    nc.scalar.activation(
                        gact[:],
                        ps[:],
                        TANH if g == 2 else SIG,
                        bias=bias_sb[:, g * M_TILES + mt : g * M_TILES + mt + 1],
                    )
                    gt[g] = gact

                for g in range(4):
                    ps = banks[g][0 if d == 0 else 1]
                    kf8 = kf8_by_dir[d][g]
                    for kh in range(kf8, KH):
                        nc.tensor.matmul(
                            ps[:],
                            whm[:, g, kh - KH_MIN8, :],
                            h_sb[d][:, kh - kf8min_d[d], nsl],
                            start=False,
                            stop=(kf8 == 0 and kh == KH - 1),
                            skip_group_check=True,
                        )
                    if kf8 == 0:
                        act(g)
                for g in range(4):  # hidden DoubleRow run
                    kf8 = kf8_by_dir[d][g]
                    if kf8 == 0:
                        continue
                    for p in range(kf8 // 2):
                        nc.tensor.matmul(
                            ps := banks[g][0 if d == 0 else 1],
                            wh8m[:, g, 2 * p : 2 * p + 2, :],
                            h8_sb[d][:, 2 * p : 2 * p + 2, nsl],
                            start=False,
                            stop=(p == kf8 // 2 - 1),
                            skip_group_check=True,
                            perf_mode=DR,
                        )
                    act(g)
                return gt

            def tail_dir(mt, n, d, gt, cpd):
                """Cell-update front half for one direction: ig/fc/cw on
                DVE, tanh on ScalarE. Direction 0's runs mid-block (its
                gates finish after dir0's matmuls); direction 1's runs at
                the START of the next block - only 3 DVE ops then sit ahead
                of the px copies, which stay inside the px window. (GpSimd
                stays idle: 1.4us/op, and an early all-GpSimd variant
                coincided with a chip-wide ~1.2x downclock.)"""
                ws, iws = wss[d], inv_ws[d]
                ig = t_pool.tile([128, NCH], MM_DT, tag="ig", name=f"ig{d}")
                nc.vector.scalar_tensor_tensor(ig[:], gt[0][:], ws, gt[2][:], MULT, MULT)
                fc = t_pool.tile([128, NCH], MM_DT, tag="fc", name=f"fc{d}")
                nc.vector.tensor_mul(fc[:], gt[1][:], cpd[:])
                cwd = dr_pool.tile([128, NCH], MM_DT, tag="cw", name=f"cw{d}")
                nc.vector.tensor_add(cwd[:], ig[:], fc[:])
                tchd = t_pool.tile([128, NCH], MM_DT, tag="tch", name=f"tch{d}")
                nc.scalar.activation(tchd[:], cwd[:], TANH, scale=iws)
                return cwd, tchd

            def tail_fin(mt, n, gt_d, cp, msl, cw, tch):
                """Cell-update back half: hw, direction combine, stores."""
                nsl = slice(n * NCH, (n + 1) * NCH)
                hw = []
                for d in (0, 1):
                    hwd = dr_pool.tile([128, NCH], MM_DT, tag="hw", name=f"hw{d}")
                    nc.vector.scalar_tensor_tensor(
                        hwd[:], gt_d[d][3][:], wss[d], tch[d][:], MULT, MULT
                    )
                    hw.append(hwd)
                # combine + store; output triggers ride the scalar queue
                ctt = o_pool.tile([128, NCH], MM_DT, tag="ctt")
                nc.vector.tensor_add(ctt[:], cw[0][:], cw[1][:])
                nc.scalar.dma_start(ctd[msl, nsl], ctt[:])
                htt = o_pool.tile([128, NCH], MM_DT, tag="htt")
                nc.vector.tensor_add(htt[:], hw[0][:], hw[1][:])
                nc.scalar.dma_start(htd[msl, nsl], htt[:])

            pending = None  # (mt, n, gt_d, cp, msl, cw0, tch0)
            for mt in range(M_TILES):
                msl = slice(mt * 128, (mt + 1) * 128)
                if mt + 2 < M_TILES:
                    load_w(mt + 2)
                wxm = wx_tiles.pop(mt)
                whm = wh_tiles.pop(mt)
                wh8m = wh8_tiles.pop(mt) if wh8d is not None else None
                wx8m = wx8_tiles.pop(mt) if wx8d is not None else None

                for n in range(N_CHUNKS):
                    # c_prev loads (host pre-scaled by ws_d) ride the SCALAR
                    # hw queue (small store transfers only): on Sync they
                    # trigger behind the multi-MB weight prefetches, land
                    # ~10us late, and their wait then head-of-line-blocks
                    # the DVE FIFO ahead of the px copies
                    nsl = slice(n * NCH, (n + 1) * NCH)
                    cp = [
                        cp_pool.tile([128, NCH], MM_DT, tag="cp", name=f"cp_{mt}_{n}_{d}")
                        for d in (0, 1)
                    ]
                    nc.scalar.dma_start(cp[0][:], cd_[0][msl, nsl])
                    nc.scalar.dma_start(cp[1][:], cd_[1][msl, nsl])

                    banks = px_phase(mt, n, wxm, wx8m, recompute=(mt == 0 and n == 0))
                    if pending is not None:
                        p_mt, p_n, p_gt, p_cp, p_msl, p_cw0, p_tch0 = pending
                        # NOTE: the ~0.6us/block stall of the first in-place
                        # direction-0 matmul (its copy WAR wait is hoisted
                        # onto prefetched LDWEIGHTS, so it effectively waits
                        # for the g1 copy too) resists reordering: the
                        # scheduler is work-conserving and dispatches this
                        # elementwise (ready at block start) ahead of the
                        # copies; tc.high_priority() doesn't override
                        # readiness order and tc.tile_wait_until() emits
                        # real runtime waits (measured +7us). Accepted.
                        cw1, tch1 = tail_dir(p_mt, p_n, 1, p_gt[1], p_cp[1])
                    # direction 0 in place on the px banks, 1 on the clones
                    gt0 = dir_mms(mt, n, 0, banks, whm, wh8m)
                    cw0, tch0 = tail_dir(mt, n, 0, gt0, cp[0])
                    gt1 = dir_mms(mt, n, 1, banks, whm, wh8m)
                    if pending is not None:
                        tail_fin(p_mt, p_n, p_gt, p_cp, p_msl,
                                 [p_cw0, cw1], [p_tch0, tch1])
                    pending = (mt, n, [gt0, gt1], cp, msl, cw0, tch0)
            p_mt, p_n, p_gt, p_cp, p_msl, p_cw0, p_tch0 = pending
            cw1, tch1 = tail_dir(p_mt, p_n, 1, p_gt[1], p_cp[1])
            tail_fin(p_mt, p_n, p_gt, p_cp, p_msl, [p_cw0, cw1], [p_tch0, tch1])

    nc.finalize()
    n_mm = sum(
        1 for i in nc.inst_map.values() if type(i).__name__ == "InstMatmult"
    )
    expected_mm = 4 * KX + M_TILES * N_CHUNKS * sum(
        (KX - KX8_G[g]) + KX8_G[g] // 2
        + sum((KH - kf8_by_dir[d][g]) + kf8_by_dir[d][g] // 2 for d in (0, 1))
        for g in range(4)
    )
    assert n_mm == expected_mm, f"matmul count {n_mm} != {expected_mm}"
    return nc


_CACHE: dict = {}


def _get_nc(ws0: float, ws1: float):
    key = (ws0, ws1)
    if key not in _CACHE:
        _CACHE.clear()
        _CACHE[key] = _build(ws0, ws1)
    return _CACHE[key]


def _prep_w(w: np.ndarray, ktiles, np_dt) -> np.ndarray:
    """(OUT_C, K) weight -> [m_tile, partition, k_tile, m_in_tile] lhsT tiles
    for the given k-tile indices."""
    wT = np.ascontiguousarray(w.T)  # (K, OUT_C)
    r = wT.reshape(-1, 128, M_TILES, 128)  # [ktile, p, mtile, mi]
    r = r[list(ktiles)]
    return np.ascontiguousarray(r.transpose(2, 1, 0, 3).astype(np_dt))


def _prep_wstack(ws: list[np.ndarray], ktiles, np_dt) -> np.ndarray:
    """4 gate weights -> [m_tile, partition, gate, k_tile, m_in_tile]."""
    s = np.stack([_prep_w(w, ktiles, np_dt) for w in ws])  # [g, mt, p, kt, mi]
    return np.ascontiguousarray(s.transpose(1, 2, 0, 3, 4))


def _prep_rhs(a: np.ndarray, k0: int, k1: int, np_dt) -> np.ndarray:
    """(K, n) activation k-tiles [k0,k1) -> [partition, k_tile, n]."""
    r = a[k0 * 128 : k1 * 128].reshape(k1 - k0, 128, -1).transpose(1, 0, 2)
    return np.ascontiguousarray(r.astype(np_dt))


def run(inputs: dict, trace: bool = False, trace_kwargs: dict | None = None):
    x = np.asarray(inputs["x"], dtype=np.float32)
    ws = np.asarray(inputs["weighted_sum"], dtype=np.float32)
    ws0, ws1 = float(ws[0]), float(ws[1])
    nc = _get_nc(ws0, ws1)

    lo_first = abs(ws0) <= abs(ws1)
    kf8_by_dir = [KF8_LO_G, KF8_HI_G] if lo_first else [KF8_HI_G, KF8_LO_G]
    kf8max_d = [max(kf8_by_dir[d]) for d in (0, 1)]
    kf8min_d = [min(kf8_by_dir[d]) for d in (0, 1)]

    wx_list = [np.asarray(inputs[k], dtype=np.float32)
               for k in ("w_ii", "w_if", "w_ig", "w_io")]
    wh_list = [np.asarray(inputs[k], dtype=np.float32)
               for k in ("w_hi", "w_hf", "w_hg", "w_ho")]
    wx_host = _prep_wstack(wx_list, range(KX), MM_NP)
    wh_host = _prep_wstack(wh_list, range(KH_MIN8, KH), MM_NP)
    wh8_host = (
        _prep_wstack(wh_list, range(KF8MAX), F8_NP) if KF8MAX > 0 else None
    )
    wx8_host = (
        _prep_wstack(wx_list, range(KX8MAX), F8_NP) if KX8MAX > 0 else None
    )
    bias_host = np.concatenate(
        [np.asarray(inputs[k], dtype=np.float32).reshape(M_TILES, 128).T
         for k in ("b_i", "b_f", "b_g", "b_o")],
        axis=1,
    )
    bias_host = np.ascontiguousarray(bias_host)

    h = [np.asarray(inputs["h_prev_dim0"], dtype=np.float32),
         np.asarray(inputs["h_prev_dim1"], dtype=np.float32)]
    # c_prev is pre-scaled by the direction weight on the host; the kernel
    # computes cw_d = ws_d*c_d directly and ct = cw_0 + cw_1.
    c = [(np.asarray(inputs["c_prev_dim0"], dtype=np.float32) * ws0).astype(MM_NP),
         (np.asarray(inputs["c_prev_dim1"], dtype=np.float32) * ws1).astype(MM_NP)]

    in_maps = []
    for core in range(N_CORES):
        csl = slice(core * NS, (core + 1) * NS)
        m = {
            "x": _prep_rhs(x[:, csl], 0, KX, MM_NP),
            "c0": np.ascontiguousarray(c[0][:, csl]),
            "c1": np.ascontiguousarray(c[1][:, csl]),
            "wx": wx_host,
            "wh": wh_host,
            "bias": bias_host,
        }
        if wh8_host is not None:
            m["wh8"] = wh8_host
        if wx8_host is not None:
            m["wx8"] = wx8_host
            m["x8"] = _prep_rhs(x[:, csl], 0, KX8MAX, F8_NP)
        for d in (0, 1):
            if kf8min_d[d] < KH:
                m[f"h{d}"] = _prep_rhs(h[d][:, csl], kf8min_d[d], KH, MM_NP)
            if kf8max_d[d] > 0:
                m[f"h8{d}"] = _prep_rhs(h[d][:, csl], 0, kf8max_d[d], F8_NP)
        in_maps.append(m)

    res = run_bass_kernel_spmd(
        nc,
        in_maps,
        list(range(N_CORES)),
        trace=trace,
        **(trace_kwargs or {}),
    )
    ct = np.concatenate(
        [res.results[c]["ct"].astype(np.float32) for c in range(N_CORES)], axis=1
    )
    ht = np.concatenate(
        [res.results[c]["ht"].astype(np.float32) for c in range(N_CORES)], axis=1
    )
    return (ct, ht), res


def kernel(**inputs) -> tuple:
    (ct, ht), _ = run(inputs)
    return ct, ht


# revision 43
# speedup vs baseline: 1.1915x; 1.1906x over previous
"""MDLSTM cell (2-direction) Bass/Tile kernel for Trainium2, 8-core SPMD.

Math (per direction d, with shared input projections):
    i = sigmoid(w_ii @ x + w_hi @ h_d + b_i)
    f = sigmoid(w_if @ x + w_hf @ h_d + b_f)
    g = tanh   (w_ig @ x + w_hg @ h_d + b_g)
    o = sigmoid(w_io @ x + w_ho @ h_d + b_o)
    c_d = f * c_prev_d + i * g
    h_d = o * tanh(c_d)
ct = ws0 * c_0 + ws1 * c_1 ;  ht = ws0 * h_0 + ws1 * h_1

Sharding: all activations/states split along N (=8192) across 8 cores;
weights replicated. No cross-core communication.

Per-core kernel, one n-chunk (512 cols) at a time: per gate the shared
x-projection accumulates in a PSUM bank (start=True group); a DVE
PSUM->PSUM copy clones it into a second bank as direction 1's base, then
direction 0's hidden projection accumulates IN PLACE on the px bank
(start=False matmuls - no inject needed) and direction 1's accumulates on
the copy. For the very first block, direction 1's base is RECOMPUTED
(start=True) instead of copied: only TensorE writes set PSUM has_written
bits, and a start=False accumulate onto a DVE-written virgin bank would
overwrite instead of accumulate - the recompute also fills the DMA-bound
startup with useful work. ScalarE applies sigmoid/tanh + per-partition
bias straight out of PSUM; the elementwise cell update runs on DVE
(GpSimd measured 1.4us/op, and an all-GpSimd variant coincided with a
chip-wide ~1.2x downclock), split per direction and software-pipelined
one block behind the matmuls so the px copies stay near the DVE queue
head (the in-place direction-0 matmuls carry a WAR dependency on them).

Precision: matmul operands are fp16 (same PE rate as bf16, 3 extra
mantissa bits of error headroom) except fp8e4m3 DoubleRow k-tiles (2
k-tiles per instruction, batched into contiguous runs - the first DR
after a non-DR matmul pays ~190ns of exposed LDWEIGHTS, back-to-back DRs
hide it and issue at the same 216ns cadence as one bf16 k-tile).
Per-(direction,gate) fp8 k-tile counts are hardcoded from an offline
ml_dtypes error model (bit-faithful: matched HW rel_fro to 4 digits at
three calibration points); the low-|ws| direction runs fully fp8 - its
quantization error is attenuated by the weighted sum - and the tanh (g)
gate is the most error-sensitive, so the high-|ws| direction keeps it in
fp16. Measured rel_fro 1.874e-2 vs the 2e-2 gate. c_prev loads and ct/ht
stores are fp16 (upcast on host). The direction weighted sum is folded
in algebraically: c_prev is pre-scaled by ws_d on the host, (i*ws)*g
runs as one scalar_tensor_tensor, tanh(c_d) is recovered from the
weighted cw via activation input scale 1/ws_d, and the final combine is
two adds.

Startup is DMA-ramp-bound (~8.5us before the first payload packet):
weights are laid out with all 4 gates contiguous per partition row and
x k-tile 0 rides at the head of the Activation engine's HW-DGE queue
(it ramps ~2us before Sync's); the first block's px runs k-tile-major so
the first 8 matmuls need only half of x. Later weight prefetches go via
Sync; c_prev loads and output stores trigger from the scalar queue -
a DMA trigger blocks its engine queue until the previous transfer on the
same hw queue completes, so small transfers must not share a queue with
the multi-MB weight prefetches.
"""

import numpy as np

import concourse.bass as bass  # noqa: F401  (bass types via bacc/tile)
import concourse.mybir as mybir
import concourse.tile as tile
from concourse import bacc
from concourse.bass_utils import run_bass_kernel_spmd

N_CORES = 8
IN_C = 512
OUT_C = 1024
N = 8192
NS = N // N_CORES  # columns per core
NCH = 512  # psum free-dim chunk (one bank)
N_CHUNKS = NS // NCH
KX = IN_C // 128  # k-tiles of the input projection
KH = OUT_C // 128  # k-tiles of the hidden projection
M_TILES = OUT_C // 128

F32 = mybir.dt.float32
MM_MODE = "fp16"  # one of: "fp32r", "bf16", "fp16"
import ml_dtypes as _mld
MM_DT = {"fp32r": mybir.dt.float32r, "bf16": mybir.dt.bfloat16,
         "fp16": mybir.dt.float16}[MM_MODE]
MM_NP = {"fp32r": np.float32, "bf16": _mld.bfloat16,
         "fp16": np.float16}[MM_MODE]

SIG = mybir.ActivationFunctionType.Sigmoid
TANH = mybir.ActivationFunctionType.Tanh
MULT = mybir.AluOpType.mult
ADD = mybir.AluOpType.add
COPY = mybir.ActivationFunctionType.Copy

# fp8 k-tile counts (must be even - DoubleRow packs 2 k-tiles/instruction).
# _LO applies to the direction with the smaller |ws| (its error is
# attenuated by the weighted sum), _HI to the other. Per-gate order
# [i, f, g, o]. KX8 is the shared x projection (error hits both dirs).
# Chosen by the offline ml_dtypes error model (err_model.py greedy, fp16
# high path): predicted rel_fro 1.875e-2 vs the 2e-2 gate (model matched
# HW to 4 digits at two calibration points; inputs are deterministic).
KF8_LO_G = [8, 8, 8, 8]
KF8_HI_G = [8, 8, 0, 4]
KX8_G = [0, 0, 0, 0]
KF8MAX = max(max(KF8_LO_G), max(KF8_HI_G))
KH_MIN8 = min(min(KF8_LO_G), min(KF8_HI_G))  # bf16 wh skips these k-tiles
KX8MAX = max(KX8_G)
F8 = mybir.dt.float8e4
F8_NP = _mld.float8_e4m3fn
DR = mybir.MatmulPerfMode.DoubleRow


def _build(ws0: float, ws1: float):
    nc = bacc.Bacc(
        "TRN2", target_bir_lowering=False, debug=False, num_devices=N_CORES
    )

    lo_first = abs(ws0) <= abs(ws1)
    kf8_by_dir = [KF8_LO_G, KF8_HI_G] if lo_first else [KF8_HI_G, KF8_LO_G]
    kf8max_d = [max(kf8_by_dir[d]) for d in (0, 1)]
    kf8min_d = [min(kf8_by_dir[d]) for d in (0, 1)]

    xd = nc.dram_tensor("x", [128, KX, NS], MM_DT, kind="ExternalInput")
    x8d = (
        nc.dram_tensor("x8", [128, KX8MAX, NS], F8, kind="ExternalInput")
        if KX8MAX > 0 else None
    )
    # bf16 h only carries the k-tiles the bf16 path actually reads
    # (kf8min_d..KH-1); the first k-tiles live only in the fp8 copies.
    hd_ = [
        nc.dram_tensor(
            f"h{d}", [128, KH - kf8min_d[d], NS], MM_DT, kind="ExternalInput",
        ) if kf8min_d[d] < KH else None
        for d in (0, 1)
    ]
    cd_ = [
        nc.dram_tensor(f"c{d}", [OUT_C, NS], MM_DT, kind="ExternalInput")
        for d in (0, 1)
    ]
    # weights: [m_tile, partition(k%128), gate, k_tile, m_in_tile] - all 4
    # gates contiguous per partition row so one DMA moves 4-8KB runs
    # (startup is DMA packet-rate-bound, not byte-bound). bf16 wh skips
    # k-tiles 0..KH_MIN8-1 (fp8 in every direction/gate).
    wxd = nc.dram_tensor("wx", [M_TILES, 128, 4, KX, 128], MM_DT, kind="ExternalInput")
    wx8d = (
        nc.dram_tensor("wx8", [M_TILES, 128, 4, KX8MAX, 128], F8, kind="ExternalInput")
        if KX8MAX > 0 else None
    )
    whd = nc.dram_tensor(
        "wh", [M_TILES, 128, 4, KH - KH_MIN8, 128], MM_DT, kind="ExternalInput"
    )
    wh8d = (
        nc.dram_tensor(
            "wh8", [M_TILES, 128, 4, KF8MAX, 128], F8, kind="ExternalInput"
        ) if KF8MAX > 0 else None
    )
    h8d_ = [
        nc.dram_tensor(
            f"h8{d}", [128, kf8max_d[d], NS], F8, kind="ExternalInput"
        ) if kf8max_d[d] > 0 else None
        for d in (0, 1)
    ]
    biasd = nc.dram_tensor("bias", [128, 4 * M_TILES], F32, kind="ExternalInput")
    # outputs stored bf16 (upcast on host): halves the store traffic that
    # gates the kernel-end barrier; adds only ~0.2% rms to the outputs
    ctd = nc.dram_tensor("ct", [OUT_C, NS], MM_DT, kind="ExternalOutput")
    htd = nc.dram_tensor("ht", [OUT_C, NS], MM_DT, kind="ExternalOutput")

    # reciprocal scales for the tanh(c) recovery; clamp so ws==0 stays finite
    # (then cw==0 and tanh(0)==0 gives the right answer anyway)
    inv_ws = [1.0 / max(ws0, 1e-20), 1.0 / max(ws1, 1e-20)]
    wss = [ws0, ws1]

    with tile.TileContext(nc) as tc:
        with (
            tc.tile_pool(name="resident", bufs=1) as res_pool,
            tc.tile_pool(name="wx", bufs=3) as wx_pool,
            tc.tile_pool(name="wh", bufs=3) as wh_pool,
            tc.tile_pool(name="wh8", bufs=3) as wh8_pool,
            tc.tile_pool(name="psum", bufs=8, space="PSUM") as ps_pool,
            tc.tile_pool(name="gates", bufs=16) as g_pool,
            tc.tile_pool(name="cprev", bufs=4) as cp_pool,
            tc.tile_pool(name="tmp", bufs=8) as t_pool,
            tc.tile_pool(name="dirres", bufs=6) as dr_pool,
            tc.tile_pool(name="out", bufs=4) as o_pool,
        ):
            wx_tiles: dict = {}
            wx8_tiles: dict = {}
            wh_tiles: dict = {}
            wh8_tiles: dict = {}

            def alloc_w(mt):
                wx_tiles[mt] = wx_pool.tile(
                    [128, 4, KX, 128], MM_DT, tag="wx", name=f"wx_{mt}"
                )
                wh_tiles[mt] = wh_pool.tile(
                    [128, 4, KH - KH_MIN8, 128], MM_DT, tag="wh", name=f"wh_{mt}"
                )
                if wh8d is not None:
                    wh8_tiles[mt] = wh8_pool.tile(
                        [128, 4, KF8MAX, 128], F8, tag="wh8", name=f"wh8_{mt}"
                    )
                if wx8d is not None:
                    wx8_tiles[mt] = wx8_pool.tile(
                        [128, 4, KX8MAX, 128], F8, tag="wx8", name=f"wx8_{mt}"
                    )

            def load_w(mt, eng=None):
                # startup loads ride the Activation engine's HW-DGE queue so
                # the trigger path ramps in parallel with Sync's x/h loads;
                # mid-kernel prefetches go via Sync - a DMA trigger can block
                # its queue, and the scalar queue's ACTs free PSUM banks.
                eng = eng or nc.sync
                alloc_w(mt)
                eng.dma_start(wx_tiles[mt][:], wxd[mt])
                eng.dma_start(wh_tiles[mt][:], whd[mt])
                if wh8d is not None:
                    eng.dma_start(wh8_tiles[mt][:], wh8d[mt])
                if wx8d is not None:
                    eng.dma_start(wx8_tiles[mt][:], wx8d[mt])

            x_sb = res_pool.tile([128, KX, NS], MM_DT, tag="x")
            x8_sb = (
                res_pool.tile([128, KX8MAX, NS], F8, tag="x8")
                if x8d is not None else None
            )
            h_sb = [
                res_pool.tile(
                    [128, KH - kf8min_d[d], NS], MM_DT,
                    tag=f"h{d}", name=f"h_sb{d}",
                ) if hd_[d] is not None else None
                for d in (0, 1)
            ]
            bias_sb = res_pool.tile([128, 4 * M_TILES], F32, tag="bias")

            # Startup is DMA packet-rate-bound: full-tensor loads give
            # 8-16KB contiguous runs (vs 1KB chunked), cutting packet count
            # ~8x. Order by first use; wx0 split per gate-pair so px g0/g1
            # can start before the full gate set lands.
            alloc_w(0)
            # The two tensors MM #1 needs are split ACROSS the two HW-DGE
            # queues so they transfer in parallel (the 16 DMA engines
            # round-robin between queues): x k-tile 0 heads the scalar
            # queue (ramps ~2us before Sync's), wx0 gates 0-1 head Sync's.
            # Measured: both on one queue serialized and gated MM #1 at
            # ~14.9us.
            nc.scalar.dma_start(x_sb[:, 0:1], xd[:, 0:1])
            nc.scalar.dma_start(wx_tiles[0][:, 2:4], wxd[0][:, 2:4])
            if wx8d is not None:
                nc.scalar.dma_start(wx8_tiles[0][:], wx8d[0])
            # wh8 before wh: direction 0 (all-fp8) runs right after px
            if wh8d is not None:
                nc.scalar.dma_start(wh8_tiles[0][:], wh8d[0])
            nc.scalar.dma_start(wh_tiles[0][:], whd[0])
            nc.sync.dma_start(wx_tiles[0][:, 0:2], wxd[0][:, 0:2])
            h8_sb = [
                res_pool.tile(
                    [128, kf8max_d[d], NS], F8,
                    tag=f"h8{d}", name=f"h8_sb{d}",
                ) if h8d_[d] is not None else None
                for d in (0, 1)
            ]
            # x in two k-tile halves (same layout, 4KB runs): the first px
            # matmuls only need k-tiles 0-1, shaving the critical startup
            # bytes before the first MM
            nc.sync.dma_start(x_sb[:, 1:2], xd[:, 1:2])
            nc.sync.dma_start(x_sb[:, 2:3], xd[:, 2:3])
            nc.sync.dma_start(x_sb[:, 3:4], xd[:, 3:4])
            if x8_sb is not None:
                nc.sync.dma_start(x8_sb[:], x8d[:])
            # activation loads ordered by first use: fp8 h first (the all-
            # fp8 direction's DR run follows px immediately), bias before
            # the first gate ACT, the big fp16 h in halves so the first
            # bf16 hidden matmuls need not wait for the full tensor
            if h8_sb[0] is not None:
                nc.sync.dma_start(h8_sb[0][:], h8d_[0][:])
            nc.sync.dma_start(bias_sb[:], biasd[:])
            for d in (0, 1):
                if h_sb[d] is not None:
                    kt_n = KH - kf8min_d[d]
                    if kt_n >= 2:
                        nc.sync.dma_start(h_sb[d][:, : kt_n // 2], hd_[d][:, : kt_n // 2])
                        nc.sync.dma_start(h_sb[d][:, kt_n // 2 :], hd_[d][:, kt_n // 2 :])
                    else:
                        nc.sync.dma_start(h_sb[d][:], hd_[d][:])
                if d == 1 and h8_sb[d] is not None:
                    nc.sync.dma_start(h8_sb[d][:], h8d_[d][:])
            # NOT on the scalar queue: a DMA trigger blocks its queue until
            # the previous transfer on the same hw queue completes, and the
            # scalar queue must stay free for the ACTs the phases wait on.
            load_w(1)

            def px_phase(mt, n, wxm, wx8m, recompute=False):
                """Per gate: x-projection into a PSUM bank (start=True) and
                a DVE clone of it into a second bank (direction 1's base).
                fp8 k-tiles of all gates run as one contiguous DoubleRow run
                (a DR instruction after a non-DR one pays ~183ns of exposed
                LDWEIGHTS; back-to-back DR ones hide it).
                Returns [(px_bank, dir1_bank), ...] per gate."""
                nsl = slice(n * NCH, (n + 1) * NCH)
                banks = []
                for g in range(4):
                    px = ps_pool.tile(
                        [128, NCH], F32, tag="ps", name=f"px_{mt}_{n}_{g}"
                    )
                    banks.append([px, None])
                # First block: k-tile-major, so the first 8 matmuls need
                # only the first half of x (lands first). Elsewhere:
                # gate-major, so gate g's bank completes early and its DVE
                # clone is ready before direction 0 reaches that bank.
                order = (
                    [(kt, g) for kt in range(KX) for g in range(4)]
                    if recompute else
                    [(kt, g) for g in range(4) for kt in range(KX)]
                )
                for kt, g in order:
                    kx8 = KX8_G[g]
                    if kt < kx8:
                        continue
                    nc.tensor.matmul(
                        banks[g][0][:],
                        wxm[:, g, kt, :],
                        x_sb[:, kt, nsl],
                        start=(kt == kx8),
                        stop=(kt == KX - 1 and kx8 == 0),
                    )
                for g in range(4):  # px DoubleRow run
                    kx8 = KX8_G[g]
                    for p in range(kx8 // 2):
                        nc.tensor.matmul(
                            banks[g][0][:],
                            wx8m[:, g, 2 * p : 2 * p + 2, :],
                            x8_sb[:, 2 * p : 2 * p + 2, nsl],
                            start=(kx8 == KX and p == 0),
                            stop=(p == kx8 // 2 - 1),
                            skip_group_check=True,
                            perf_mode=DR,
                        )
                for g in range(4):
                    b1 = ps_pool.tile(
                        [128, NCH], F32, tag="ps", name=f"b1_{mt}_{n}_{g}"
                    )
                    if recompute:
                        # First block only: recompute the x-projection into
                        # direction 1's banks instead of cloning. This puts a
                        # start=True group on every PSUM bank before any
                        # DVE-write + start=False accumulate happens (only
                        # TensorE writes set the has_written bits; without
                        # them those matmuls would overwrite, not accumulate)
                        # and fills the DMA-bound startup with useful work.
                        for kt in range(KX):
                            nc.tensor.matmul(
                                b1[:],
                                wxm[:, g, kt, :],
                                x_sb[:, kt, nsl],
                                start=(kt == 0),
                                stop=(kt == KX - 1),
                            )
                    else:
                        # high_priority: the Tile scheduler orders engine
                        # queues by its own heap; the copies must land ahead
                        # of the previous block's elementwise on DVE or the
                        # in-place direction-0 matmuls (WAR on the copy
                        # read) stall ~0.6us per block
                        with tc.high_priority():
                            nc.vector.tensor_copy(b1[:], banks[g][0][:])
                    banks[g][1] = b1
                return banks

            def dir_mms(mt, n, d, banks, whm, wh8m):
                """Hidden-projection accumulation for one direction onto the
                already-seeded PSUM banks (bank index: 0 = in place on px,
                1 = the DVE clone), then the gate activations. All gates'
                bf16 k-tiles first, then every fp8 pair in one contiguous
                DoubleRow run; each gate's ACT follows its last matmul."""
                nsl = slice(n * NCH, (n + 1) * NCH)
                gt = [None] * 4

                def act(g):
                    ps = banks[g][0 if d == 0 else 1]
                    gact = g_pool.tile(
                        [128, NCH], MM_DT, tag="gate", name=f"gate_{mt}_{n}_{d}_{g}"
                    )
                    nc.scalar.activation(
                        gact[:],
                        ps[:],
                        TANH if g == 2 else SIG,
                        bias=bias_sb[:, g * M_TILES + mt : g * M_TILES + mt + 1],
                    )
                    gt[g] = gact

                for g in range(4):
                    ps = banks[g][0 if d == 0 else 1]
                    kf8 = kf8_by_dir[d][g]
                    for kh in range(kf8, KH):
                        nc.tensor.matmul(
                            ps[:],
                            whm[:, g, kh - KH_MIN8, :],
                            h_sb[d][:, kh - kf8min_d[d], nsl],
                            start=False,
                            stop=(kf8 == 0 and kh == KH - 1),
                            skip_group_check=True,
                        )
                    if kf8 == 0:
                        act(g)
                for g in range(4):  # hidden DoubleRow run
                    kf8 = kf8_by_dir[d][g]
                    if kf8 == 0:
                        continue
                    for p in range(kf8 // 2):
                        nc.tensor.matmul(
                            ps := banks[g][0 if d == 0 else 1],
                            wh8m[:, g, 2 * p : 2 * p + 2, :],
                            h8_sb[d][:, 2 * p : 2 * p + 2, nsl],
                            start=False,
                            stop=(p == kf8 // 2 - 1),
                            skip_group_check=True,
                            perf_mode=DR,
                        )
                    act(g)
                return gt

            def tail_dir(mt, n, d, gt, cpd):
                """Cell-update front half for one direction: ig/fc/cw on
                DVE, tanh on ScalarE. Direction 0's runs mid-block (its
                gates finish after dir0's matmuls); direction 1's runs at
                the START of the next block - only 3 DVE ops then sit ahead
                of the px copies, which stay inside the px window. (GpSimd
                stays idle: 1.4us/op, and an early all-GpSimd variant
                coincided with a chip-wide ~1.2x downclock.)"""
                ws, iws = wss[d], inv_ws[d]
                ig = t_pool.tile([128, NCH], MM_DT, tag="ig", name=f"ig{d}")
                nc.vector.scalar_tensor_tensor(ig[:], gt[0][:], ws, gt[2][:], MULT, MULT)
                fc = t_pool.tile([128, NCH], MM_DT, tag="fc", name=f"fc{d}")
                nc.vector.tensor_mul(fc[:], gt[1][:], cpd[:])
                cwd = dr_pool.tile([128, NCH], MM_DT, tag="cw", name=f"cw{d}")
                nc.vector.tensor_add(cwd[:], ig[:], fc[:])
                tchd = t_pool.tile([128, NCH], MM_DT, tag="tch", name=f"tch{d}")
                nc.scalar.activation(tchd[:], cwd[:], TANH, scale=iws)
                return cwd, tchd

            def tail_fin(mt, n, gt_d, cp, msl, cw, tch):
                """Cell-update back half: hw, direction combine, stores."""
                nsl = slice(n * NCH, (n + 1) * NCH)
                hw = []
                for d in (0, 1):
                    hwd = dr_pool.tile([128, NCH], MM_DT, tag="hw", name=f"hw{d}")
                    nc.vector.scalar_tensor_tensor(
                        hwd[:], gt_d[d][3][:], wss[d], tch[d][:], MULT, MULT
                    )
                    hw.append(hwd)
                # combine + store; output triggers ride the scalar queue
                ctt = o_pool.tile([128, NCH], MM_DT, tag="ctt")
                nc.vector.tensor_add(ctt[:], cw[0][:], cw[1][:])
                nc.scalar.dma_start(ctd[msl, nsl], ctt[:])
                htt = o_pool.tile([128, NCH], MM_DT, tag="htt")
                nc.vector.tensor_add(htt[:], hw[0][:], hw[1][:])
                nc.scalar.dma_start(htd[msl, nsl], htt[:])

            pending = None  # (mt, n, gt_d, cp, msl, cw0, tch0)
            for mt in range(M_TILES):
                msl = slice(mt * 128, (mt + 1) * 128)
                if mt + 2 < M_TILES:
                    load_w(mt + 2)
                wxm = wx_tiles.pop(mt)
                whm = wh_tiles.pop(mt)
                wh8m = wh8_tiles.pop(mt) if wh8d is not None else None
                wx8m = wx8_tiles.pop(mt) if wx8d is not None else None

                for n in range(N_CHUNKS):
                    # c_prev loads (host pre-scaled by ws_d) ride the SCALAR
                    # hw queue (small store transfers only): on Sync they
                    # trigger behind the multi-MB weight prefetches, land
                    # ~10us late, and their wait then head-of-line-blocks
                    # the DVE FIFO ahead of the px copies
                    nsl = slice(n * NCH, (n + 1) * NCH)
                    cp = [
                        cp_pool.tile([128, NCH], MM_DT, tag="cp", name=f"cp_{mt}_{n}_{d}")
                        for d in (0, 1)
                    ]
                    nc.scalar.dma_start(cp[0][:], cd_[0][msl, nsl])
                    nc.scalar.dma_start(cp[1][:], cd_[1][msl, nsl])

                    banks = px_phase(mt, n, wxm, wx8m, recompute=(mt == 0 and n == 0))
                    if pending is not None:
                        p_mt, p_n, p_gt, p_cp, p_msl, p_cw0, p_tch0 = pending
                        # NOTE: the ~0.6us/block stall of the first in-place
                        # direction-0 matmul (its copy WAR wait is hoisted
                        # onto prefetched LDWEIGHTS, so it effectively waits
                        # for the g1 copy too) resists reordering: the
                        # scheduler is work-conserving and dispatches this
                        # elementwise (ready at block start) ahead of the
                        # copies; tc.high_priority() doesn't override
                        # readiness order and tc.tile_wait_until() emits
                        # real runtime waits (measured +7us). Accepted.
                        cw1, tch1 = tail_dir(p_mt, p_n, 1, p_gt[1], p_cp[1])
                    # direction 0 in place on the px banks, 1 on the clones
                    gt0 = dir_mms(mt, n, 0, banks, whm, wh8m)
                    cw0, tch0 = tail_dir(mt, n, 0, gt0, cp[0])
                    gt1 = dir_mms(mt, n, 1, banks, whm, wh8m)
                    if pending is not None:
                        tail_fin(p_mt, p_n, p_gt, p_cp, p_msl,
                                 [p_cw0, cw1], [p_tch0, tch1])
                    pending = (mt, n, [gt0, gt1], cp, msl, cw0, tch0)
            # Final block's tail at HALF width, two pipelined halves: this
            # chain runs after the last matmul and is latency-bound across
            # alternating DVE/ScalarE hops, so halving the op width shortens
            # the critical path (half 1's vector ops overlap half 0's tanh).
            p_mt, p_n, p_gt, p_cp, p_msl, p_cw0, p_tch0 = pending
            ws1_, iws1 = wss[1], inv_ws[1]
            NH = NCH // 2
            for hi in range(2):
                h = slice(hi * NH, (hi + 1) * NH)
                nslh = slice(p_n * NCH + hi * NH, p_n * NCH + (hi + 1) * NH)
                gt1 = p_gt[1]
                ig = t_pool.tile([128, NH], MM_DT, tag="ig", name=f"figh{hi}")
                nc.vector.scalar_tensor_tensor(
                    ig[:], gt1[0][:, h], ws1_, gt1[2][:, h], MULT, MULT
                )
                fc = t_pool.tile([128, NH], MM_DT, tag="fc", name=f"ffch{hi}")
                nc.vector.tensor_mul(fc[:], gt1[1][:, h], p_cp[1][:, h])
                cw1 = dr_pool.tile([128, NH], MM_DT, tag="cw", name=f"fcwh{hi}")
                nc.vector.tensor_add(cw1[:], ig[:], fc[:])
                tch1 = t_pool.tile([128, NH], MM_DT, tag="tch", name=f"ftchh{hi}")
                nc.scalar.activation(tch1[:], cw1[:], TANH, scale=iws1)
                hw0 = dr_pool.tile([128, NH], MM_DT, tag="hw", name=f"fhw0h{hi}")
                nc.vector.scalar_tensor_tensor(
                    hw0[:], p_gt[0][3][:, h], wss[0], p_tch0[:, h], MULT, MULT
                )
                hw1 = dr_pool.tile([128, NH], MM_DT, tag="hw", name=f"fhw1h{hi}")
                nc.vector.scalar_tensor_tensor(
                    hw1[:], gt1[3][:, h], ws1_, tch1[:], MULT, MULT
                )
                ctt = o_pool.tile([128, NH], MM_DT, tag="ctt", name=f"fctth{hi}")
                nc.vector.tensor_add(ctt[:], p_cw0[:, h], cw1[:])
                nc.scalar.dma_start(ctd[p_msl, nslh], ctt[:])
                htt = o_pool.tile([128, NH], MM_DT, tag="htt", name=f"fhtth{hi}")
                nc.vector.tensor_add(htt[:], hw0[:], hw1[:])
                nc.scalar.dma_start(htd[p_msl, nslh], htt[:])

    nc.finalize()
    n_mm = sum(
        1 for i in nc.inst_map.values() if type(i).__name__ == "InstMatmult"
    )
    expected_mm = 4 * KX + M_TILES * N_CHUNKS * sum(
        (KX - KX8_G[g]) + KX8_G[g] // 2
        + sum((KH - kf8_by_dir[d][g]) + kf8_by_dir[d][g] // 2 for d in (0, 1))
        for g in range(4)
    )
    assert n_mm == expected_mm, f"matmul count {n_mm} != {expected_mm}"
    return nc


_CACHE: dict = {}


def _get_nc(ws0: float, ws1: float):
    key = (ws0, ws1)
    if key not in _CACHE:
        _CACHE.clear()
        _CACHE[key] = _build(ws0, ws1)
    return _CACHE[key]


def _prep_w(w: np.ndarray, ktiles, np_dt) -> np.ndarray:
    """(OUT_C, K) weight -> [m_tile, partition, k_tile, m_in_tile] lhsT tiles
    for the given k-tile indices."""
    wT = np.ascontiguousarray(w.T)  # (K, OUT_C)
    r = wT.reshape(-1, 128, M_TILES, 128)  # [ktile, p, mtile, mi]
    r = r[list(ktiles)]
    return np.ascontiguousarray(r.transpose(2, 1, 0, 3).astype(np_dt))


def _prep_wstack(ws: list[np.ndarray], ktiles, np_dt) -> np.ndarray:
    """4 gate weights -> [m_tile, partition, gate, k_tile, m_in_tile]."""
    s = np.stack([_prep_w(w, ktiles, np_dt) for w in ws])  # [g, mt, p, kt, mi]
    return np.ascontiguousarray(s.transpose(1, 2, 0, 3, 4))


def _prep_rhs(a: np.ndarray, k0: int, k1: int, np_dt) -> np.ndarray:
    """(K, n) activation k-tiles [k0,k1) -> [partition, k_tile, n]."""
    r = a[k0 * 128 : k1 * 128].reshape(k1 - k0, 128, -1).transpose(1, 0, 2)
    return np.ascontiguousarray(r.astype(np_dt))


def run(inputs: dict, trace: bool = False, trace_kwargs: dict | None = None):
    x = np.asarray(inputs["x"], dtype=np.float32)
    ws = np.asarray(inputs["weighted_sum"], dtype=np.float32)
    ws0, ws1 = float(ws[0]), float(ws[1])
    nc = _get_nc(ws0, ws1)

    lo_first = abs(ws0) <= abs(ws1)
    kf8_by_dir = [KF8_LO_G, KF8_HI_G] if lo_first else [KF8_HI_G, KF8_LO_G]
    kf8max_d = [max(kf8_by_dir[d]) for d in (0, 1)]
    kf8min_d = [min(kf8_by_dir[d]) for d in (0, 1)]

    wx_list = [np.asarray(inputs[k], dtype=np.float32)
               for k in ("w_ii", "w_if", "w_ig", "w_io")]
    wh_list = [np.asarray(inputs[k], dtype=np.float32)
               for k in ("w_hi", "w_hf", "w_hg", "w_ho")]
    wx_host = _prep_wstack(wx_list, range(KX), MM_NP)
    wh_host = _prep_wstack(wh_list, range(KH_MIN8, KH), MM_NP)
    wh8_host = (
        _prep_wstack(wh_list, range(KF8MAX), F8_NP) if KF8MAX > 0 else None
    )
    wx8_host = (
        _prep_wstack(wx_list, range(KX8MAX), F8_NP) if KX8MAX > 0 else None
    )
    bias_host = np.concatenate(
        [np.asarray(inputs[k], dtype=np.float32).reshape(M_TILES, 128).T
         for k in ("b_i", "b_f", "b_g", "b_o")],
        axis=1,
    )
    bias_host = np.ascontiguousarray(bias_host)

    h = [np.asarray(inputs["h_prev_dim0"], dtype=np.float32),
         np.asarray(inputs["h_prev_dim1"], dtype=np.float32)]
    # c_prev is pre-scaled by the direction weight on the host; the kernel
    # computes cw_d = ws_d*c_d directly and ct = cw_0 + cw_1.
    c = [(np.asarray(inputs["c_prev_dim0"], dtype=np.float32) * ws0).astype(MM_NP),
         (np.asarray(inputs["c_prev_dim1"], dtype=np.float32) * ws1).astype(MM_NP)]

    in_maps = []
    for core in range(N_CORES):
        csl = slice(core * NS, (core + 1) * NS)
        m = {
            "x": _prep_rhs(x[:, csl], 0, KX, MM_NP),
            "c0": np.ascontiguousarray(c[0][:, csl]),
            "c1": np.ascontiguousarray(c[1][:, csl]),
            "wx": wx_host,
            "wh": wh_host,
            "bias": bias_host,
        }
        if wh8_host is not None:
            m["wh8"] = wh8_host
        if wx8_host is not None:
            m["wx8"] = wx8_host
            m["x8"] = _prep_rhs(x[:, csl], 0, KX8MAX, F8_NP)
        for d in (0, 1):
            if kf8min_d[d] < KH:
                m[f"h{d}"] = _prep_rhs(h[d][:, csl], kf8min_d[d], KH, MM_NP)
            if kf8max_d[d] > 0:
                m[f"h8{d}"] = _prep_rhs(h[d][:, csl], 0, kf8max_d[d], F8_NP)
        in_maps.append(m)

    res = run_bass_kernel_spmd(
        nc,
        in_maps,
        list(range(N_CORES)),
        trace=trace,
        **(trace_kwargs or {}),
    )
    ct = np.concatenate(
        [res.results[c]["ct"].astype(np.float32) for c in range(N_CORES)], axis=1
    )
    ht = np.concatenate(
        [res.results[c]["ht"].astype(np.float32) for c in range(N_CORES)], axis=1
    )
    return (ct, ht), res


def kernel(**inputs) -> tuple:
    (ct, ht), _ = run(inputs)
    return ct, ht
